# revision 1
# baseline (speedup 1.0000x reference)
"""Trainium2 Bass kernel for CrossAttentionModel.

Model (per batch element b, done per-core; 8 cores = data-parallel over B):
  q = relu(STE_Q @ Wq + bq) * (1/8 folded into Wq,bq host-side)
  k = relu(STE_P @ Wk + bk);  v = relu(X @ Wv + bv)
  per (node n, head h):  S = q_h K_h^T ; P = softmax(S, axis=tp)
  out = relu((P V) @ W1 + b1) @ W2 + b2

On-chip layout strategy (per core):
  tokens t = tq*256 + nn (natural).  Per block of NB=8 nodes, load the
  A-rows for all 48 tq of those nodes (partition-strided DMA), PE-transpose
  to get A^T tiles [d, u] (u = local (nn,tq) index), run all projections,
  attention (block-diagonal head-pair packed matmuls), MLP, and write the
  final natural-layout output.  fp32r (tf32-class) for the 512-contraction
  matmuls, exact fp32 for the attention matmuls.
"""
import sys

sys.path.insert(0, "/opt/trn_rl_repo")

from contextlib import ExitStack

import numpy as np

import concourse.bacc as bacc
import concourse.bass as bass
import concourse.tile as tile
from concourse import mybir
from concourse.bass_utils import run_bass_kernel_spmd
from concourse.masks import make_identity

F32 = mybir.dt.float32
DTM = mybir.dt.float32r  # reduced-precision matmul dtype for big matmuls

B, T, NN, D = 8, 48, 256, 512
H, DH = 8, 64
TOK = T * NN            # 12288 tokens per core
NB = 8                  # nodes per block
NBLK = NN // NB         # 32 blocks
U = NB * T              # 384 u-tokens per block
NCORES = 8
DEBUG = False

_CACHE = {}


def _build():
    nc = bacc.Bacc("TRN2", target_bir_lowering=False, debug=False,
                   num_devices=NCORES)

    ins = {}
    for name in ("steq", "step", "xv"):
        ins[name] = nc.dram_tensor(name, [TOK, D], F32, kind="ExternalInput").ap()
    for name in ("wq", "wk", "wv", "w1", "w2"):
        ins[name] = nc.dram_tensor(name, [D, D], F32, kind="ExternalInput").ap()
    for name in ("bq", "bk", "bv", "b1", "b2"):
        ins[name] = nc.dram_tensor(name, [D], F32, kind="ExternalInput").ap()
    out_d = nc.dram_tensor("out", [TOK, D], F32, kind="ExternalOutput").ap()
    dbg = {}
    if DEBUG:
        for name, shape in (("d_xt", [128, U]), ("d_qt", [128, U]),
                            ("d_kt", [128, U]), ("d_v", [96, D]),
                            ("d_p", [96, NB, 4, T]), ("d_rec", [96, 2 * 4 * T]),
                            ("d_pn", [96, NB, 4, T]), ("d_ot", [128, U]),
                            ("d_ktbd", [128, 4, 96]), ("d_vbd", [96, D]),
                            ("d_a", [96, D]), ("d_psx", [128, U])):
            dbg[name] = nc.dram_tensor(name, shape, F32,
                                       kind="ExternalOutput").ap()

    with tile.TileContext(nc) as tc:
        with ExitStack() as ctx:
            _body(ctx, tc, ins, out_d, dbg)
    nc.compile()
    return nc


def _body(ctx, tc, ins, out_d, dbg=None):
    nc = tc.nc

    # ---------------- pools ----------------
    singles = ctx.enter_context(tc.tile_pool(name="singles", bufs=1))
    wload = ctx.enter_context(tc.tile_pool(name="wload", bufs=1))
    a_pool = ctx.enter_context(tc.tile_pool(name="a", bufs=6))
    xt_pool = ctx.enter_context(tc.tile_pool(name="xt", bufs=4))
    qkv_pool = ctx.enter_context(tc.tile_pool(name="qkv", bufs=4))
    attn_pool = ctx.enter_context(tc.tile_pool(name="attn", bufs=1))
    mlp_pool = ctx.enter_context(tc.tile_pool(name="mlp", bufs=4))
    ps_wide = ctx.enter_context(tc.tile_pool(name="psw", bufs=3, space="PSUM"))
    ps_attn = ctx.enter_context(tc.tile_pool(name="psa", bufs=3, space="PSUM"))
    ps_ot = ctx.enter_context(tc.tile_pool(name="pso", bufs=2, space="PSUM"))
    dram_pool = ctx.enter_context(tc.tile_pool(name="dscr", bufs=2, space="DRAM"))

    # ---------------- constants / weights (once) ----------------
    ident = singles.tile([128, 128], F32)
    make_identity(nc, ident[:])

    w_sb = {}
    for name in ("wq", "wk", "wv", "w1", "w2"):
        wtmp = wload.tile([128, 4, D], F32, tag="wtmp")
        nc.sync.dma_start(wtmp[:], ins[name].rearrange("(c p) o -> p c o", p=128))
        w = singles.tile([128, 4, D], DTM, tag=f"w_{name}")
        nc.vector.tensor_copy(w[:], wtmp[:])
        w_sb[name] = w

    bias_sb = {}
    for name in ("bq", "bk", "b1"):
        bt = singles.tile([128, 4], F32, tag=f"b_{name}")
        nc.sync.dma_start(bt[:], ins[name].rearrange("(c p) -> p c", p=128))
        bias_sb[name] = bt
    # b2 broadcast across partitions [128, 512]
    b2_bc = singles.tile([128, D], F32)
    nc.sync.dma_start(b2_bc[:],
                      ins["b2"][None, :].broadcast_to((128, D)))
    # bv as a [1, 512] fp32r row (added via K=1 matmul)
    bv_f = singles.tile([1, D], F32)
    nc.sync.dma_start(bv_f[:], ins["bv"][None, :])
    bv_r = singles.tile([1, D], DTM)
    nc.vector.tensor_copy(bv_r[:], bv_f[:])
    ones1_f = singles.tile([1, 96], F32)
    nc.vector.memset(ones1_f[:], 1.0)
    ones1 = singles.tile([1, 96], DTM)
    nc.vector.tensor_copy(ones1[:], ones1_f[:])
    # [96, 2] half-indicator (col hh sums partitions of half hh), fp32r.
    # obd[p, y] = 1 iff 0 <= p - 48*y < 48 ; built with two affine selects
    # (compute engines can't address a partition base of 48 directly).
    obd_f = singles.tile([96, 2], F32)
    nc.vector.memset(obd_f[:], 1.0)
    nc.gpsimd.affine_select(out=obd_f[:], in_=obd_f[:],
                            compare_op=mybir.AluOpType.is_ge, fill=0.0,
                            base=0, pattern=[[-48, 2]], channel_multiplier=1)
    nc.gpsimd.affine_select(out=obd_f[:], in_=obd_f[:],
                            compare_op=mybir.AluOpType.is_ge, fill=0.0,
                            base=47, pattern=[[48, 2]], channel_multiplier=-1)
    obd2 = singles.tile([96, 2], DTM)
    nc.vector.tensor_copy(obd2[:], obd_f[:])

    # persistent zero-padded block-diagonal staging buffers (2 ring slots,
    # half-block = 4 nodes each).  Zeros written once; DMAs below only ever
    # overwrite the nonzero quadrants, so the zero lanes stay zero.
    ktbd = singles.tile([128, 2, 4, 4, 96], F32)   # [d, slot, nn4, c, (hh,tp)]
    vbd = singles.tile([96, 2, 4, D], F32)         # [(hh,tp), slot, nn4, (c,dd)]
    nc.gpsimd.memset(ktbd[:], 0.0)
    nc.gpsimd.memset(vbd[:], 0.0)

    src_nt = {k: ins[k].rearrange("(tq nn) d -> nn tq d", nn=NN)
              for k in ("steq", "step", "xv")}
    out_nt = out_d.rearrange("(tq nn) d -> nn tq d", nn=NN)

    for blk in range(NBLK):
        nn0 = blk * NB

        # ---- load A rows u-grouped + transpose to XT [d, u] ----
        xts = {}
        for key in ("steq", "step", "xv"):
            a_tiles = []
            for j in range(4):  # 2-node groups
                at = a_pool.tile([96, D], F32, tag="a_in")
                for sub in range(2):
                    nc.sync.dma_start(
                        at[sub * 48:(sub + 1) * 48, :],
                        src_nt[key][nn0 + 2 * j + sub, :, :])
                a_tiles.append(at)
            if DEBUG and blk == 0 and key == "steq":
                nc.sync.dma_start(dbg["d_a"][:], a_tiles[0][:])
            xt = []
            for i in range(4):
                psx = ps_wide.tile([128, U], F32, tag="mmwide")
                for j in range(4):
                    nc.tensor.matmul(psx[:, j * 96:(j + 1) * 96],
                                     a_tiles[j][:, i * 128:(i + 1) * 128],
                                     ident[0:96, 0:96], is_transpose=True)
                xt_i = xt_pool.tile([128, U], DTM, tag=f"xt_{key}")
                nc.vector.tensor_copy(xt_i[:], psx[:])
                if DEBUG and blk == 0 and key == "steq" and i == 0:
                    fcp = xt_pool.tile([128, U], F32, tag="dbgf")
                    nc.scalar.copy(fcp[:], psx[:])
                    nc.sync.dma_start(dbg["d_psx"][:], fcp[:])
                xt.append(xt_i)
            xts[key] = xt

        # ---- projections ----
        qt_sb, kt_sb = [], []
        for key, wname, bname, dst in (("steq", "wq", "bq", qt_sb),
                                       ("step", "wk", "bk", kt_sb)):
            for oc in range(4):
                ps = ps_wide.tile([128, U], F32, tag="mmwide")
                for ic in range(4):
                    nc.tensor.matmul(
                        ps[:], w_sb[wname][:, ic, oc * 128:(oc + 1) * 128],
                        xts[key][ic][:], start=(ic == 0), stop=(ic == 3))
                t = qkv_pool.tile([128, U], F32, tag=f"t_{wname}")
                nc.scalar.activation(t[:], ps[:],
                                     mybir.ActivationFunctionType.Relu,
                                     bias=bias_sb[bname][:, oc:oc + 1])
                dst.append(t)

        v_sb = []
        for us in range(4):  # 96-token (2-node) slices
            ps = ps_wide.tile([96, D], F32, tag="mmwide")
            for ic in range(4):
                nc.tensor.matmul(ps[:], xts["xv"][ic][:, us * 96:(us + 1) * 96],
                                 w_sb["wv"][:, ic, :], start=(ic == 0), stop=False)
            nc.tensor.matmul(ps[:], ones1[:], bv_r[:], start=False, stop=True)
            vt = qkv_pool.tile([96, D], F32, tag="t_v")
            nc.scalar.activation(vt[:], ps[:], mybir.ActivationFunctionType.Relu)
            v_sb.append(vt)

        # ---- stage block-diagonal KT / V ----
        for half in range(2):
            slot = (2 * blk + half) % 2
            for c in range(4):
                for hh in range(2):
                    nc.sync.dma_start(
                        ktbd[hh * 64:(hh + 1) * 64, slot, :, c,
                             hh * 48:(hh + 1) * 48],
                        kt_sb[c][hh * 64:(hh + 1) * 64,
                                 half * 192:(half + 1) * 192]
                        .rearrange("p (n t) -> p n t", n=4))
            for jj in range(2):  # v_sb tiles in this half (2-node each)
                vt = v_sb[half * 2 + jj]
                for sub in range(2):  # node within tile
                    nnl4 = jj * 2 + sub
                    for hh in range(2):
                        nc.sync.dma_start(
                            vbd[hh * 48:(hh + 1) * 48, slot, nnl4, :]
                            .rearrange("p (c e) -> p c e", c=4)
                            [:, :, hh * 64:(hh + 1) * 64],
                            vt[sub * 48:(sub + 1) * 48, :]
                            .rearrange("p (c e) -> p c e", c=4)
                            [:, :, hh * 64:(hh + 1) * 64])

        # ---- attention ----
        p_blk = attn_pool.tile([96, NB, 4, T], F32, tag="p")      # exp(scores)
        pn_blk = attn_pool.tile([96, NB, 4, T], F32, tag="pn")    # normalized
        rec_g = attn_pool.tile([96, 4, 8], F32, tag="recg")       # spread denoms
        for pair in range(4):
            for half_nn in range(2):
                nnl = 2 * pair + half_nn
                slot = (2 * blk + nnl // 4) % 2
                pss = ps_attn.tile([96, 4, T], F32, tag="attn")
                for c in range(4):
                    nc.tensor.matmul(pss[:, c, :],
                                     ktbd[:, slot, nnl % 4, c, :],
                                     qt_sb[c][:, nnl * T:(nnl + 1) * T])
                nc.scalar.activation(p_blk[:, nnl], pss[:],
                                     mybir.ActivationFunctionType.Exp)
        pr_blk = attn_pool.tile([96, NB, 4, T], DTM, tag="pr")
        nc.vector.tensor_copy(pr_blk[:], p_blk[:])
        for pair in range(4):
            psd = ps_attn.tile([2, 2 * 4 * T], F32, tag="attn")
            nc.tensor.matmul(psd[:], obd2[:],
                             pr_blk[:, 2 * pair:2 * pair + 2]
                             .rearrange("p a c t -> p (a c t)"))
            psd_sb = attn_pool.tile([2, 2 * 4 * T], F32, tag="psdsb")
            nc.scalar.copy(psd_sb[:], psd[:])
            # spread 2x384 distinct denominators over 96 partitions x 8
            for hh in range(2):
                nc.sync.dma_start(
                    rec_g[hh * 48:(hh + 1) * 48, pair, :],
                    psd_sb[hh:hh + 1, :].rearrange("o (cg e) -> o cg e", e=8))
        rec_r = attn_pool.tile([96, 4, 8], F32, tag="recr")
        nc.vector.reciprocal(rec_r[:], rec_g[:])
        # bounce through DRAM so the replication below can use a 0-step
        # (partition-broadcast) source AP, which SBUF sources don't allow
        rec_d = dram_pool.tile([96, 4, 8], F32, tag="recd")
        nc.sync.dma_start(rec_d[:], rec_r[:])
        for pair in range(4):
            rec_b = attn_pool.tile([96, 2 * 4 * T], F32, tag="recb")
            # replicate: dst[(hh,tp), col] = rec[(hh, col//8), pair, col%8]
            for hh in range(2):
                nc.sync.dma_start(
                    rec_b[hh * 48:(hh + 1) * 48, :]
                    .rearrange("tp (cg e) -> tp cg e", e=8),
                    rec_d[hh * 48:(hh + 1) * 48, pair, :][None, :, :]
                    .broadcast_to((48, 48, 8)))
            if DEBUG and blk == 0 and pair == 0:
                nc.sync.dma_start(dbg["d_rec"][:], rec_b[:])
            nc.vector.tensor_mul(
                pn_blk[:, 2 * pair:2 * pair + 2].rearrange("p a c t -> p (a c t)"),
                p_blk[:, 2 * pair:2 * pair + 2].rearrange("p a c t -> p (a c t)"),
                rec_b[:])

        ot_sb = []
        for c in range(4):
            pso = ps_ot.tile([128, NB, T], F32, tag="ot")
            for nnl in range(NB):
                slot = (2 * blk + nnl // 4) % 2
                nc.tensor.matmul(pso[:, nnl, :],
                                 vbd[:, slot, nnl % 4,
                                     c * 128:(c + 1) * 128],
                                 pn_blk[:, nnl, c, :])
            ot = mlp_pool.tile([128, U], DTM, tag="ot")
            nc.vector.tensor_copy(ot[:], pso[:].rearrange("p n t -> p (n t)"))
            ot_sb.append(ot)

        # ---- MLP ----
        if DEBUG and blk == 0:
            nc.sync.dma_start(dbg["d_xt"][:],
                              xts["steq"][0][:].bitcast(F32))
            nc.sync.dma_start(dbg["d_qt"][:], qt_sb[0][:])
            nc.sync.dma_start(dbg["d_kt"][:], kt_sb[0][:])
            nc.sync.dma_start(dbg["d_v"][:], v_sb[0][:])
            nc.sync.dma_start(dbg["d_p"][:], p_blk[:])
            nc.sync.dma_start(dbg["d_pn"][:], pn_blk[:])
            nc.sync.dma_start(dbg["d_ot"][:], ot_sb[0][:].bitcast(F32))
            nc.sync.dma_start(dbg["d_ktbd"][:], ktbd[:, 0, 0, :, :])
            nc.sync.dma_start(dbg["d_vbd"][:], vbd[:, 0, 0, :])

        mt_sb = []
        for mc in range(4):
            ps = ps_wide.tile([128, U], F32, tag="mmwide")
            for dc in range(4):
                nc.tensor.matmul(ps[:], w_sb["w1"][:, dc, mc * 128:(mc + 1) * 128],
                                 ot_sb[dc][:], start=(dc == 0), stop=(dc == 3))
            mt = mlp_pool.tile([128, U], DTM, tag="mt")
            nc.scalar.activation(mt[:], ps[:],
                                 mybir.ActivationFunctionType.Relu,
                                 bias=bias_sb["b1"][:, mc:mc + 1])
            mt_sb.append(mt)

        for us in range(3):  # 128-token slices of the block
            ps = ps_wide.tile([128, D], F32, tag="mmwide")
            for mc in range(4):
                nc.tensor.matmul(ps[:], mt_sb[mc][:, us * 128:(us + 1) * 128],
                                 w_sb["w2"][:, mc, :], start=(mc == 0),
                                 stop=(mc == 3))
            fo = mlp_pool.tile([128, D], F32, tag="fout")
            nc.vector.tensor_add(fo[:], ps[:], b2_bc[:])
            # natural-layout output rows t = tq*256 + nn ; split at node bounds
            u0 = us * 128
            r = 0
            while r < 128:
                ul = u0 + r
                nnl, tq0 = ul // T, ul % T
                ln = min(T - tq0, 128 - r)
                nc.sync.dma_start(
                    out_nt[nn0 + nnl, tq0:tq0 + ln, :], fo[r:r + ln, :])
                r += ln


def kernel(**inputs):
    if "nc" not in _CACHE:
        _CACHE["nc"] = _build()
    nc = _CACHE["nc"]

    X = np.asarray(inputs["X"], dtype=np.float32)
    SP = np.asarray(inputs["STE_P"], dtype=np.float32)
    SQ = np.asarray(inputs["STE_Q"], dtype=np.float32)
    scale = np.float32(1.0 / np.sqrt(DH))
    shared = {
        "wq": np.ascontiguousarray(inputs["Wq"], dtype=np.float32) * scale,
        "bq": np.ascontiguousarray(inputs["bq"], dtype=np.float32) * scale,
        "wk": np.ascontiguousarray(inputs["Wk"], dtype=np.float32),
        "bk": np.ascontiguousarray(inputs["bk"], dtype=np.float32),
        "wv": np.ascontiguousarray(inputs["Wv"], dtype=np.float32),
        "bv": np.ascontiguousarray(inputs["bv"], dtype=np.float32),
        "w1": np.ascontiguousarray(inputs["W1"], dtype=np.float32),
        "b1": np.ascontiguousarray(inputs["b1"], dtype=np.float32),
        "w2": np.ascontiguousarray(inputs["W2"], dtype=np.float32),
        "b2": np.ascontiguousarray(inputs["b2"], dtype=np.float32),
    }
    in_maps = []
    for b in range(NCORES):
        m = dict(shared)
        m["steq"] = np.ascontiguousarray(SQ[b].reshape(TOK, D))
        m["step"] = np.ascontiguousarray(SP[b].reshape(TOK, D))
        m["xv"] = np.ascontiguousarray(X[b].reshape(TOK, D))
        in_maps.append(m)

    _CACHE["in_maps"] = in_maps
    res = run_bass_kernel_spmd(nc, in_maps, list(range(NCORES)))
    out = np.stack([res.results[b]["out"].reshape(T, NN, D)
                    for b in range(NCORES)])
    return out

